# revision 30
# baseline (speedup 1.0000x reference)
"""DepthWeightedAssignment Trainium2 kernel (v5: host-packed moving data).

Per-detection argmin over 64 cameras of
  cost[i,j] = (d_i-c_j)^2 + 0.5*(1-exp(-0.045 c_j)) + 0.3*(t_i-t_j)^2/3600
sharded over 8 NeuronCores (N axis); input packing, thresholds and weights
on host.

Device algorithm (per core, N_C = 131072 detections):
  - One PE matmul per 512-column chunk computes the full encoded key
      X = 2^23 + 64*k + j,   k ~ round(768*cost)
    directly in PSUM.  The PE accumulates 32-row bands of the K axis in row
    order internally (verified empirically), so each detection gets its own
    32-row band holding its complete sequence
      [18 bf16-split data rows, A0..A2, +2^30 (quantizes S=49152*cost to the
       64*k grid at f32 ulp), -(2^30-2^23) (exact bf16 restore), +j]
    with the other detection's band zeroed on these stationary columns;
    cross-band combination adds 0, exact under any ordering.  K = 56:
    band L rows 0-23 (stationary cols 0-63 = cams), pad rows 24-31,
    band H rows 32-55 (cols 64-127).  PSUM: 128 partitions =
    (detL cams0-31 | detL cams32-63 | detH cams0-31 | detH cams32-63).
  - DVE tensor_reduce(min, axis=X, apply_transpose=True) reads PSUM directly
    and reduces each 32-camera partition group into the free dim; X is
    order-preserving in cost with j as tiebreak, exact for every row whose
    min cost is far below the 625 threshold (the ones that matter).
  - Results stream to DRAM as a [128, 2048] f32 tile; host combines the two
    32-camera groups, unpacks j = X & 63, k = (X >> 6) - 2^17, applies the
    threshold and computes weights in float64.
The bf16 triple-split coefficient rows (18 per detection: d-100 paired with
B splits, (d-100)^2 with SS, sw*(t-1800) with C splits, its square with SS)
are precomputed on the host and shipped as a packed [18, N_C] bf16 input, so
the device does no data preparation at all: DMA in, matmul, reduce, DMA out.
"""
import sys

sys.path.insert(0, "/opt/trn_rl_repo")

import numpy as np

N_TOTAL = 1 << 20
M_CAMS = 64
N_CORES = 8
N_C = N_TOTAL // N_CORES          # 131072 per core
CH = 8192                         # moving columns per M-tile
NQ = N_C // (2 * CH)              # 8 M-tiles per core (L+H windows per tile)
REG = 2048                        # psum region columns
NREG = CH // REG                  # 4 regions per M-tile
S_FINE = 768.0
SS = 64.0 * S_FINE                # 49152
H_BIAS = 2.0 ** 23
H_QUANT = 2.0 ** 30
H_REST = -(2.0 ** 30 - 2.0 ** 23)
W_T = 0.3 / 3600.0
DC = 100.0                        # depth centering
TCEN = 1800.0                     # time centering
THRESH_K = 625.0 * S_FINE         # 480000
K_BIAS = int(H_BIAS) >> 6         # 131072
KROWS = 56                        # K rows: band L 0-23, pad 24-31, band H 32-55

_CACHE = {}


def _build_module():
    import concourse.bacc as bacc
    import concourse.tile as tile
    from concourse import mybir

    f32 = mybir.dt.float32
    bf16 = mybir.dt.bfloat16
    OP = mybir.AluOpType
    AX = mybir.AxisListType

    nc = bacc.Bacc("TRN2", target_bir_lowering=False)

    mdata = nc.dram_tensor("mdata", [KROWS, N_C // 2], bf16,
                           kind="ExternalInput")
    stat_in = nc.dram_tensor("stat", [KROWS, 128], bf16, kind="ExternalInput")
    x_out = nc.dram_tensor("xout", [128, 2048], f32, kind="ExternalOutput")

    with tile.TileContext(nc) as tc:
        with (
            tc.tile_pool(name="const", bufs=1) as cpool,
            tc.tile_pool(name="mov", bufs=3) as mpool,
            tc.tile_pool(name="ps", bufs=2, space="PSUM") as ppool,
            tc.tile_pool(name="s1", bufs=2) as s1pool,
        ):
            stat_t = cpool.tile([KROWS, 128], bf16)
            nc.sync.dma_start(stat_t[:], stat_in[:])

            # PE clock warmup: keep the tensor engine continuously busy on
            # throwaway matmuls so the p-state ramp completes before the
            # first real region.  A memset junk tile is ready ~1.4us before
            # the stat DMA lands, so the ramp starts immediately.
            junk = cpool.tile([KROWS, 128], bf16)
            nc.gpsimd.memset(junk[:], 1.0)
            warm = ppool.tile([128, REG], f32, tag="ps")
            for _ in range(30):
                nc.tensor.matmul(warm[:, 0:128], junk[:], junk[:],
                                 start=True, stop=True)

            for q in range(NQ):
                m = mpool.tile([KROWS, CH], bf16, tag="m")
                # mdata is host-packed in final m-tile layout: rows 0-31 =
                # L-window data + ones/pad, rows 32-55 = H-window data +
                # ones.  One DMA per M-tile; q=0 loads the first psum
                # region's columns separately to shorten the head.
                c0 = q * CH
                if q == 0:
                    nc.sync.dma_start(m[:, 0:REG], mdata[:, c0:c0 + REG])
                    nc.sync.dma_start(m[:, REG:CH],
                                      mdata[:, c0 + REG:c0 + CH])
                else:
                    nc.sync.dma_start(m[:], mdata[:, c0:c0 + CH])

                s1 = s1pool.tile([128, 256], f32, tag="s1")
                for r in range(NREG):
                    ps = ppool.tile([128, REG], f32, tag="ps")
                    for c in range(REG // 512):
                        col = r * REG + c * 512
                        nc.tensor.matmul(
                            ps[:, c * 512:(c + 1) * 512],
                            stat_t[:],
                            m[:, col:col + 512],
                            start=True,
                            stop=True,
                        )
                    nc.vector.tensor_reduce(
                        out=s1[:, r * 64:(r + 1) * 64],
                        in_=ps[:].rearrange("p (b j) -> p b j", j=32),
                        op=OP.min,
                        axis=AX.X,
                        apply_transpose=True,
                    )
                if q == NQ - 1:   # split the last writes to shrink the tail
                    nc.scalar.dma_start(
                        x_out[:, q * 256:q * 256 + 128], s1[:, 0:128])
                    nc.scalar.dma_start(
                        x_out[:, q * 256 + 128:q * 256 + 192],
                        s1[:, 128:192])
                    nc.scalar.dma_start(
                        x_out[:, q * 256 + 192:(q + 1) * 256],
                        s1[:, 192:256])
                else:
                    nc.scalar.dma_start(
                        x_out[:, q * 256:(q + 1) * 256], s1[:])

    nc.compile()
    return nc


def _host_consts(camera_depths, camera_times):
    import ml_dtypes
    bf = ml_dtypes.bfloat16

    def split3(x):
        x = np.asarray(x, np.float32)
        x0 = x.astype(bf).astype(np.float32)
        r1 = (x - x0).astype(np.float32)
        x1 = r1.astype(bf).astype(np.float32)
        r2 = (r1 - x1).astype(np.float32)
        x2 = r2.astype(bf).astype(np.float32)
        return x0, x1, x2

    cd = np.asarray(camera_depths, np.float64)
    ct = np.asarray(camera_times, np.float64)
    sw = float(np.float32(np.sqrt(W_T)))
    c1 = cd - DC
    t2c = sw * ct - sw * TCEN
    L = 0.5 * (1.0 - np.exp(-0.045 * cd))
    A = (SS * (c1 * c1 + t2c * t2c + L)).astype(np.float32)
    B = (SS * (-2.0 * c1)).astype(np.float32)
    C = (SS * (-2.0 * t2c)).astype(np.float32)
    Bs, Cs, As = split3(B), split3(C), split3(A)
    jall = np.arange(64, dtype=np.float32)

    stat = np.zeros((KROWS, 128), np.float32)
    # per-band rows (matches the mdata row layout):
    # r0+0..2: d0 x (B0 B1 B2) ; +3..4: d1 x (B0 B1) ; +5: d2 x B0
    # +6..8: SS (q0 q1 q2) ; +9..11: t0 x (C0 C1 C2) ; +12..13: t1 x (C0 C1)
    # +14: t2 x C0 ; +15..17: SS (u0 u1 u2) ; +18..20: A splits ;
    # +21: +2^30 ; +22: restore ; +23: j
    for r0, cols in ((0, slice(0, 64)), (32, slice(64, 128))):
        for ri, cf in enumerate([Bs[0], Bs[1], Bs[2], Bs[0], Bs[1], Bs[0]]):
            stat[r0 + ri, cols] = cf
        stat[r0 + 6:r0 + 9, cols] = SS
        for ri, cf in enumerate([Cs[0], Cs[1], Cs[2], Cs[0], Cs[1], Cs[0]]):
            stat[r0 + 9 + ri, cols] = cf
        stat[r0 + 15:r0 + 18, cols] = SS
        stat[r0 + 18, cols] = As[0]
        stat[r0 + 19, cols] = As[1]
        stat[r0 + 20, cols] = As[2]
        stat[r0 + 21, cols] = H_QUANT
        stat[r0 + 22, cols] = H_REST
        stat[r0 + 23, cols] = jall
    return stat.astype(bf)


def _pack_mdata(dd, dt):
    """[18, N] bf16 moving-data rows for the whole problem (host prep)."""
    import ml_dtypes
    bf = ml_dtypes.bfloat16

    sw = np.float32(np.sqrt(np.float32(W_T)))
    dprime = (dd - np.float32(DC)).astype(np.float32)
    q = (dprime * dprime).astype(np.float32)
    tau = ((dt - np.float32(TCEN)) * sw).astype(np.float32)
    u = (tau * tau).astype(np.float32)

    def split3(x):
        x0 = x.astype(bf)
        r1 = (x - x0.astype(np.float32)).astype(np.float32)
        x1 = r1.astype(bf)
        x2 = (r1 - x1.astype(np.float32)).astype(np.float32).astype(bf)
        return x0, x1, x2

    ds = split3(dprime)
    qs = split3(q)
    ts = split3(tau)
    us = split3(u)
    md = np.empty((32, dd.shape[0]), bf)
    for i, v in enumerate((ds[0], ds[0], ds[0], ds[1], ds[1], ds[2],
                           qs[0], qs[1], qs[2],
                           ts[0], ts[0], ts[0], ts[1], ts[1], ts[2],
                           us[0], us[1], us[2])):
        md[i] = v
    md[18:32] = np.ones((), bf)   # band consts (x1.0 rows) + pad
    return md


def _pack_core(md_c):
    """[32, N_C] -> [56, N_C//2] final m-tile layout (one DMA per M-tile):
    per M-tile q: rows 0-31 = L window (dets 16384q..+8192), rows 32-55 =
    first 24 rows of the H window (next 8192 dets)."""
    v = md_c.reshape(32, NQ, 2, CH)
    out = np.empty((KROWS, NQ * CH), md_c.dtype)
    o3 = out.reshape(KROWS, NQ, CH)
    o3[0:32] = v[:, :, 0, :]
    o3[32:56] = v[0:24, :, 1, :]
    return out


def _det_perm():
    """(p', m) -> core-local det index, p' = 32g + j (g = cam group), m =
    256q + 64r + b; det = 16384q + 8192*(g>>1) + 2048r + 32b + j."""
    p = np.arange(128)[:, None]
    m = np.arange(2048)[None, :]
    g = p // 32
    j = p % 32
    q = m >> 8
    r = (m >> 6) & 3
    b = m & 63
    det = 16384 * q + 8192 * (g >> 1) + 2048 * r + 32 * b + j
    return det


def kernel(detection_depths, camera_depths, detection_times, camera_times):
    from concourse.bass_utils import run_bass_kernel_spmd

    if "nc" not in _CACHE:
        _CACHE["nc"] = _build_module()
        dp = _det_perm()
        _CACHE["perm"] = np.concatenate(
            [dp[0:32].ravel(), dp[64:96].ravel()])
    nc = _CACHE["nc"]
    perm = _CACHE["perm"]

    dd = np.ascontiguousarray(np.asarray(detection_depths, np.float32))
    dt = np.ascontiguousarray(np.asarray(detection_times, np.float32))
    stat = _host_consts(camera_depths, camera_times)
    md = _pack_mdata(dd, dt)

    in_maps = []
    for c in range(N_CORES):
        sl = slice(c * N_C, (c + 1) * N_C)
        in_maps.append({
            "mdata": _pack_core(md[:, sl]),
            "stat": stat,
        })
    results = run_bass_kernel_spmd(nc, in_maps, list(range(N_CORES))).results

    assignments = np.empty(N_TOTAL, np.int32)
    weights = np.empty(N_TOTAL, np.float32)
    for c in range(N_CORES):
        xo = results[c]["xout"]  # [128, 2048] f32
        xl = np.minimum(xo[0:32], xo[32:64])     # L dets, 64-cam min
        xh = np.minimum(xo[64:96], xo[96:128])   # H dets
        X = np.concatenate([xl.ravel(), xh.ravel()])
        ui = X.astype(np.int64)
        j = (ui & 63).astype(np.int32)
        kq = (ui >> 6) - K_BIAS
        np.maximum(kq, 0, out=kq)
        valid = kq < THRESH_K
        cost = kq.astype(np.float64) / S_FINE
        w = (1.0 / (1.0 + np.sqrt(cost))).astype(np.float32)
        base = c * N_C
        a_loc = np.empty(N_C, np.int32)
        w_loc = np.empty(N_C, np.float32)
        a_loc[perm] = np.where(valid, j, -1)
        w_loc[perm] = np.where(valid, w, np.float32(0.0))
        assignments[base:base + N_C] = a_loc
        weights[base:base + N_C] = w_loc
    return assignments, weights
